# revision 21
# baseline (speedup 1.0000x reference)
# Trainium2 Bass kernel for CausalStructureGAT (B=4, N=2048, D=128, H=4, C=64)
#
# Math: xt = einsum('bnd,hdc->bhnc', x, W); s_i = xt @ a_i; s_j = xt @ a_j
#       scores[b,h,i,j] = leaky_relu(s_i[i] + s_j[j], 0.2), masked where
#       causal_structure[i,j]==0; attn = softmax_j; out = attn @ xt;
#       out *= sigmoid(out @ gate_w.T + gate_b); concat heads.
#
# Key trick: softmax_j is invariant to adding any g(i) per query row. With
# g(i) = -0.2*s_i and lrelu(s) = 0.2*s + 0.8*relu(s):
#   exp(lrelu(s_i+s_j) - 0.2 s_i) = exp(0.2 s_j) * exp(0.8 relu(s_i+s_j))
#                                 = max(e^{0.8 s_i} e^{s_j}, e^{0.2 s_j})
# i.e. the (unnormalized) attention weights are max of a rank-1 outer
# product and a per-column constant -- NO N^2 exponentials or leaky-relus.
# Per score tile this is ONE DVE tensor_scalar (4x bf16 mode):
#   P1 = (U * v_j) max w_j, with U = e^{0.8 s_i} broadcast over partitions,
# plus ONE tensor_tensor multiply (2x) with the binary mask.
# The softmax denominator comes from a ones-column in the PE accumulation.
#
# Sharding: 8 cores = batch(4) x query-half(2). Layout: score tiles are
# [j on partitions, i on free] so P feeds PE directly as rhs with
# lhsT=[xt|1] (contract over j). Mask is transposed host-side, sent as
# multiplicative {0,1} bf16.
#
# Engines: DVE does the N^2 score+mask work, PE the matmuls, ACT only
# cheap 1-D exps / PSUM copies / sigmoid, Pool the epilogue multiplies.

import numpy as np
from contextlib import ExitStack

B, N, D, H, C = 4, 2048, 128, 4, 64
HALF = N // 2  # query rows per core
NCORES = 8
JT = N // 128  # 16 j-tiles

_cache = {}


def _build(repeat=1):
    import concourse.bass as bass
    import concourse.bacc as bacc
    import concourse.tile as tile
    import concourse.mybir as mybir

    f32 = mybir.dt.float32
    f32r = mybir.dt.float32r
    bf16 = mybir.dt.bfloat16
    AF = mybir.ActivationFunctionType
    OP = mybir.AluOpType

    nc = bacc.Bacc("TRN2", target_bir_lowering=False, debug=False)

    xr_d = nc.dram_tensor("xr", [D, N], f32, kind="ExternalInput").ap()
    mT_d = nc.dram_tensor("mT", [N, HALF], bf16, kind="ExternalInput").ap()
    uqh_d = nc.dram_tensor("uqh", [H, HALF], bf16, kind="ExternalInput").ap()
    vw_d = nc.dram_tensor("vw", [128, JT, 8], f32, kind="ExternalInput").ap()
    W_d = nc.dram_tensor("W", [H, D, C], f32, kind="ExternalInput").ap()
    gwT_d = nc.dram_tensor("gwT", [C, C], bf16, kind="ExternalInput").ap()
    gb_d = nc.dram_tensor("gb", [C], f32, kind="ExternalInput").ap()
    out_d = nc.dram_tensor("out", [H, 2, C, 512], f32, kind="ExternalOutput").ap()

    with tile.TileContext(nc) as tc:
        with ExitStack() as ctx:
            singles = ctx.enter_context(tc.tile_pool(name="singles", bufs=1))
            work = ctx.enter_context(tc.tile_pool(name="work", bufs=3))
            pbufp = ctx.enter_context(tc.tile_pool(name="pbufp", bufs=3))
            xtp = ctx.enter_context(tc.tile_pool(name="xtp", bufs=2))
            epi = ctx.enter_context(tc.tile_pool(name="epi", bufs=2))
            ps_acc = ctx.enter_context(
                tc.tile_pool(name="ps_acc", bufs=4, space="PSUM"))
            ps_small = ctx.enter_context(
                tc.tile_pool(name="ps_small", bufs=2, space="PSUM"))
            ps_epi = ctx.enter_context(
                tc.tile_pool(name="ps_epi", bufs=1, space="PSUM"))

            # ---- one-time params ----
            ones_j = singles.tile([1, 128], bf16)
            nc.vector.memset(ones_j, 1.0)
            ones_1 = singles.tile([1, 1], bf16)
            nc.vector.memset(ones_1, 1.0)
            ones_f = singles.tile([1, C], f32)
            nc.vector.memset(ones_f, 1.0)

            for rep in range(repeat):
                # ---- per-rep streams ----
                # U[p, h, i] = e^{0.8 s_i} broadcast over partitions via
                # a stride-0 partition read of the tiny host-computed uqh
                U = singles.tile([128, H, HALF], bf16, tag="U")
                nc.sync.dma_start(
                    out=U,
                    in_=bass.AP(tensor=uqh_d.tensor, offset=0,
                                ap=[[0, 128], [HALF, H], [1, HALF]]))
                vw = singles.tile([128, JT, 8], f32, tag="vw")
                nc.sync.dma_start(out=vw, in_=vw_d)
                xT = singles.tile([128, N], f32, tag="xT")  # x^T: [d, n]
                nc.sync.dma_start(out=xT, in_=xr_d)
                mTs = singles.tile([128, JT, HALF], bf16, tag="mTs")
                for k in range(4):
                    nc.sync.dma_start(
                        out=mTs[:, k * 4:(k + 1) * 4, :],
                        in_=mT_d.rearrange("(t p) i -> p t i", p=128)[
                            :, k * 4:(k + 1) * 4, :])
                if rep == 0:
                    W_sb = singles.tile([128, H, C], f32, tag="W_sb")
                    nc.sync.dma_start(
                        out=W_sb, in_=W_d.rearrange("h d c -> d h c"))
                    gwT_sb = singles.tile([C, C], bf16, tag="gwT_sb")
                    nc.sync.dma_start(out=gwT_sb, in_=gwT_d)
                    gb_sb = singles.tile([C, 1], f32, tag="gb_sb")
                    nc.sync.dma_start(out=gb_sb, in_=gb_d.unsqueeze(1))

                # ---- main: per head ----
                for h in range(H):
                    # xt_aug[j%128, jt, 0:C] = xt[j, :]; [.., C] = 1.0
                    xt_aug = xtp.tile([128, JT, C + 1], bf16, tag="xt_aug")
                    for k in range(JT // 4):
                        ps_xt = ps_small.tile([128, 4, C + 1], f32, tag="sm")
                        for q in range(4):
                            jt = k * 4 + q
                            nc.tensor.matmul(
                                ps_xt[:, q, :C],
                                lhsT=xT[:, jt * 128:(jt + 1) * 128],
                                rhs=W_sb[:, h, :], start=True, stop=True)
                            nc.tensor.matmul(
                                ps_xt[:, q, C:C + 1], lhsT=ones_j,
                                rhs=ones_1, start=True, stop=True)
                        nc.scalar.copy(xt_aug[:, k * 4:(k + 1) * 4, :], ps_xt)

                    # scores: P = m * max(U*v_j, w_j)   [128, JT, HALF]
                    Pb = pbufp.tile([128, JT, HALF], bf16, tag="pbuf")
                    for q in range(JT // 4):
                        t4 = work.tile([128, 4, HALF], bf16, tag="t")
                        for j4 in range(4):
                            jt = q * 4 + j4
                            nc.vector.tensor_scalar(
                                t4[:, j4, :], U[:, h, :], vw[:, jt, h:h + 1],
                                vw[:, jt, 4 + h:5 + h], op0=OP.mult, op1=OP.max)
                        nc.vector.tensor_tensor(
                            Pb[:, q * 4:(q + 1) * 4, :], t4,
                            mTs[:, q * 4:(q + 1) * 4, :], op=OP.mult)

                    # PE chases DVE tile by tile
                    acc0 = ps_acc.tile([C + 1, 512], f32, tag="acc")
                    acc1 = ps_acc.tile([C + 1, 512], f32, tag="acc")
                    acc = [acc0, acc1]
                    for jt in range(JT):
                        for ib in range(2):
                            nc.tensor.matmul(
                                acc[ib], lhsT=xt_aug[:, jt, :],
                                rhs=Pb[:, jt, ib * 512:(ib + 1) * 512],
                                start=(jt == 0), stop=(jt == JT - 1))

                    # ---- epilogue: normalize + gate + store ----
                    for ib in range(2):
                        Vc = epi.tile([C + 1, 512], f32, tag="Vc")
                        nc.scalar.copy(Vc, acc[ib])
                        r = epi.tile([1, 512], f32, tag="r")
                        nc.vector.reciprocal(r, acc[ib][C:C + 1, :])
                        ps_rb = ps_epi.tile([C, 512], f32, tag="rb")
                        nc.tensor.matmul(ps_rb, lhsT=ones_f, rhs=r,
                                         start=True, stop=True)
                        rb_sb = epi.tile([C, 512], f32, tag="rbs")
                        nc.scalar.copy(rb_sb, ps_rb)
                        Vn = epi.tile([C, 512], f32, tag="Vn")
                        nc.gpsimd.tensor_tensor(Vn, Vc[:C, :], rb_sb,
                                                op=OP.mult)
                        Vnb = epi.tile([C, 512], bf16, tag="Vnb")
                        nc.scalar.copy(Vnb, Vn)
                        ps_G = ps_epi.tile([C, 512], f32, tag="G")
                        nc.tensor.matmul(ps_G, lhsT=gwT_sb, rhs=Vnb,
                                         start=True, stop=True)
                        sg = epi.tile([C, 512], bf16, tag="sg")
                        nc.scalar.activation(sg, ps_G, AF.Sigmoid, bias=gb_sb)
                        fin = epi.tile([C, 512], f32, tag="fin")
                        nc.gpsimd.tensor_tensor(fin, Vn, sg, op=OP.mult)
                        nc.sync.dma_start(out=out_d[h, ib], in_=fin)
    nc.compile()
    return nc


def _get_program(repeat=1):
    if repeat not in _cache:
        _cache[repeat] = _build(repeat)
    return _cache[repeat]


def _host_prep(x, causal_structure, W, attention, causal_weight, gate_w, gate_b):
    import concourse.mybir as mybir
    bf16 = mybir.dt.np(mybir.dt.bfloat16)

    x = np.asarray(x, dtype=np.float32)
    causal_structure = np.asarray(causal_structure, dtype=np.float32)
    W = np.asarray(W, dtype=np.float32)
    attention = np.asarray(attention, dtype=np.float32)
    causal_weight = np.asarray(causal_weight, dtype=np.float32)
    gate_w = np.asarray(gate_w, dtype=np.float32)
    gate_b = np.asarray(gate_b, dtype=np.float32)

    a = attention[..., 0]              # (H, 2C)
    a_i, a_j = a[:, :C], a[:, C:]
    w_si = np.einsum("hdc,hc->hd", W, a_i)   # (H, D)
    w_sj = np.einsum("hdc,hc->hd", W, a_j)

    m = (causal_structure * causal_weight[0]) != 0.0   # (N_i, N_j) bool
    mT = np.ascontiguousarray(m.T.astype(bf16))        # (N_j, N_i)

    gwT = np.ascontiguousarray(gate_w.T.astype(bf16))

    in_maps = []
    for core in range(NCORES):
        b, half = core // 2, core % 2
        xr = np.ascontiguousarray(x[b].T)
        si = x[b] @ w_si.T                 # (N, H)
        sj = x[b] @ w_sj.T                 # (N, H)
        uqh = np.ascontiguousarray(
            np.exp(0.8 * si[half * HALF:(half + 1) * HALF].T).astype(bf16))
        vwf = np.concatenate([np.exp(sj), np.exp(0.2 * sj)], axis=1)  # (N, 8)
        vw = np.ascontiguousarray(
            vwf.reshape(JT, 128, 8).transpose(1, 0, 2).astype(np.float32))
        in_maps.append({
            "xr": xr,
            "mT": np.ascontiguousarray(mT[:, half * HALF:(half + 1) * HALF]),
            "uqh": uqh,
            "vw": vw,
            "W": W,
            "gwT": gwT,
            "gb": gate_b,
        })
    return in_maps


def _assemble(core_outs):
    out = np.empty((B, N, H * C), dtype=np.float32)
    for core in range(NCORES):
        b, half = core // 2, core % 2
        res = np.asarray(core_outs[core], dtype=np.float32)  # [H,2,C,512]
        out[b, half * HALF:(half + 1) * HALF, :] = \
            res.transpose(1, 3, 0, 2).reshape(HALF, H * C)
    return out


def kernel(x, causal_structure, W, attention, causal_weight, gate_w, gate_b,
           _trace=False, _repeat=1):
    from concourse.bass_utils import run_bass_kernel_spmd

    in_maps = _host_prep(x, causal_structure, W, attention, causal_weight,
                         gate_w, gate_b)
    nc = _get_program(_repeat)
    res = run_bass_kernel_spmd(nc, in_maps, list(range(NCORES)), trace=_trace)
    out = _assemble([r["out"] for r in res.results])
    if _trace:
        kernel.last_result = res
    return out
